# revision 103
# baseline (speedup 1.0000x reference)
"""OCS fused kernel for Trainium2, data-parallel over batch across 8 cores.

Algebraic restructuring (verified vs reference to ~1e-6 in fp64):

Spatial branch (4 scan orders, shared weights) collapses to a symmetric
5-point stencil with scan-order wrap rules, and the two 1x1 convs fold
through it:
    W_proj @ y_sp = A2 @ sx + (B3 - W_proj) @ x
    sx = sum of 4 flat shifts of x (+/-1 row-major, +/-w) + col-scan wraps
Channel branch: m = g g^T is rank-1, so the whole conv pipeline collapses
into three [32,128] matmuls on shifted x (weights MP/MQ/MR = u (x) P/Q/R
built on-device from g = sum of x), a silu, and one [128,32] matmul.
Diff branch: |x - nb| terms are shared between opposite directions, so one
|dx| array per axis + shifted adds gives the 4-neighbor abs-diff sum S;
W_proj folds in as W_d @ S.
BatchNorm: per-core partial (sum, sumsq) -> 1KB AllReduce -> affine.

Pipeline layout: per 2048-col group the DVE window ops, the PE channel
matmuls and the PE main matmuls overlap. gsum rides the x load (6 ACT
accum copies + 2 DVE reduces trailing the window arrivals); the channel
small-chain evacuations run on ACT so the DVE FIFO stays free for the
window wavefront; a dense matmul burst after x0 flips the PE clock gate
early; only ramp-critical weights load before x; sum(y^2) is taken from
the bf16 ypre per half-group (last two groups on the then-idle DVE);
BN stats cross 8 cores via a warmed-up 1KB AllReduce; the output is
written bf16 in four double-width 4x-mode applies on two HWDGE rings.
"""

import numpy as np
import ml_dtypes

B, C, Himg, Wimg = 8, 128, 128, 128
L = Himg * Wimg            # 16384
NCORES = 8
NCH = 512                  # psum chunk columns
NCHUNK = L // NCH          # 32
NW = 2048                  # elementwise window columns (4 chunks)
NGRP = L // NW             # 8
EPS_BN = 1e-5
NTOT = float(B * L)        # batchnorm population per channel

_CACHE = {}


def _make_patched_tc():
    """TileContext whose exit drain splits sem waits one-per-Drain.

    The walrus build in this container rejects Drain instructions carrying
    more than one sem wait ("Too many sync wait commands"). Stock
    TileContext attaches the whole global vector clock to a single tail
    Drain; emit one Drain per outstanding proc instead.
    """
    import bass_rust
    import concourse.tile as tile
    from concourse.vector_clock import ScopedClock

    class PatchedTC(tile.TileContext):
        def _drain_and_barrier(self, tick_clock, wait_clock):
            gc = list(tick_clock.global_clock)
            for i, v in enumerate(gc):
                if v:
                    single = [0] * len(gc)
                    single[i] = v
                    d = self.nc.sync.drain()
                    wait_clock.add_sem_waits(
                        d.ins, ScopedClock({None: bass_rust.VectorClock(single)})
                    )
            self.nc.all_engine_barrier()
            assert self.sems is not None
            popped = self.nc._tile_sem_poison_stack.pop()
            assert popped is self._sem_poison
            self.nc.clear_and_free_semaphores(list(self.sems.allocated().values()))
            self.nc.all_engine_barrier()

    return PatchedTC


def _split_excess_waits(nc):
    """Walrus here allows one sem wait per instruction; hoist extras onto
    same-engine NoOps inserted immediately before the instruction."""
    import bass_rust

    nid = 0
    for blk in nc.main_func.blocks:
        out = []
        for ins in blk.instructions:
            si = getattr(ins, "sync_info", None)
            waits = list(si.on_wait) if si is not None else []
            if len(waits) > 1:
                for w in waits[:-1]:
                    nid += 1
                    nop = bass_rust.InstNoOp(
                        name=f"I-waitsplit-{nid}", ins=[], outs=[])
                    nop.engine = ins.engine
                    nop.sync_info = bass_rust.SyncInfo(
                        on_wait=[w], on_update=[])
                    nc.register_instruction(nop, overwrite=True)
                    out.append(nop)
                si.on_wait = [waits[-1]]
                ins.sync_info = si
            out.append(ins)
        blk.instructions = out


def _build_program():
    import concourse.bass as bass
    import concourse.mybir as mybir

    PatchedTC = _make_patched_tc()

    f32 = mybir.dt.float32
    bf16 = mybir.dt.bfloat16
    Alu = mybir.AluOpType
    Act = mybir.ActivationFunctionType

    nc = bass.Bass(target_bir_lowering=False, num_devices=NCORES)

    x_ext = nc.declare_dram_parameter("x", [C, L], bf16, isOutput=False)
    wb3t_ext = nc.declare_dram_parameter("wb3t", [C, C], bf16, isOutput=False)
    wa2t_ext = nc.declare_dram_parameter("wa2t", [C, C], bf16, isOutput=False)
    wdt_ext = nc.declare_dram_parameter("wdt", [C, C], bf16, isOutput=False)
    c2t4_ext = nc.declare_dram_parameter("c2t4", [C, C], bf16, isOutput=False)
    wcho_ext = nc.declare_dram_parameter("wcho", [C, C], f32, isOutput=False)
    wchi_ext = nc.declare_dram_parameter("wchi", [C, C], f32, isOutput=False)
    wm1t_ext = nc.declare_dram_parameter("wm1t", [C, 32], f32, isOutput=False)
    taps_ext = nc.declare_dram_parameter("taps", [C, 3], f32, isOutput=False)
    b1t_ext = nc.declare_dram_parameter("b1t", [C, 1], f32, isOutput=False)
    bout_ext = nc.declare_dram_parameter("bout", [C, 1], f32, isOutput=False)
    gb_ext = nc.declare_dram_parameter("gb", [C, 2], f32, isOutput=False)
    y_ext = nc.declare_dram_parameter("y", [C, L], bf16, isOutput=True)

    with PatchedTC(nc) as tc:
        with (
            tc.tile_pool(name="wp", bufs=1) as wp,
            tc.tile_pool(name="big", bufs=1) as big,
            tc.tile_pool(name="win", bufs=4) as win,
            tc.tile_pool(name="dwin", bufs=2) as dwin,
            tc.tile_pool(name="sm", bufs=1) as sm,
            tc.tile_pool(name="sq", bufs=1) as sqp,
            tc.tile_pool(name="ow", bufs=4) as owp,
            tc.tile_pool(name="yps", bufs=2, space="PSUM") as yps,
            tc.tile_pool(name="hps", bufs=3, space="PSUM") as hps,
            tc.tile_pool(name="sps", bufs=1, space="PSUM") as sps,
            tc.tile_pool(name="dram", bufs=1, space="DRAM") as dram,
        ):
            # ---- weights to SBUF ----
            wb3t = wp.tile([C, C], bf16)
            wa2t = wp.tile([C, C], bf16)
            wdt = wp.tile([C, C], bf16)
            c2t4 = wp.tile([C, C], bf16)
            wcho = wp.tile([C, C], f32)
            wchi = wp.tile([C, C], f32)
            wm1t = wp.tile([C, 32], f32)
            taps = wp.tile([C, 3], f32)
            b1t = wp.tile([C, 1], f32)
            bout = wp.tile([C, 1], f32)
            gb = wp.tile([C, 2], f32)
            # only the ramp-critical weights go first (scalar ring): b1t
            # gates the ACT table prefetch, wb3t the HAM warmup matmuls.
            # The rest are issued on the sync ring AFTER the x windows so
            # their transfers don't steal SDMA slots from the x load
            # (they all land well before their first use at ~30us+).
            for t, e in [(b1t, b1t_ext), (wb3t, wb3t_ext)]:
                nc.scalar.dma_start(out=t, in_=e[:])

            ones_row = wp.tile([1, C], f32)
            nc.vector.memset(ones_row, 1.0)

            # warmup collective: pays the ~11us ncfw startup while x loads
            wu_sb = sm.tile([1, 2], f32)
            nc.vector.memset(wu_sb, 0.0)
            wu_in = dram.tile([1, 2], f32)
            wu_out = dram.tile([1, 2], f32)
            nc.sync.dma_start(out=wu_in[:], in_=wu_sb)
            nc.gpsimd.collective_compute(
                "AllReduce", Alu.add,
                replica_groups=[list(range(NCORES))],
                ins=[wu_in.opt()], outs=[wu_out.opt()])

            # ---- big SBUF arrays ----
            xbf = big.tile([C, L], bf16)     # x (bf16, cast on host)
            ypre = big.tile([C, L], bf16)    # pre-BN output (bias included)
            h1sb = big.tile([C, NGRP * NCH], bf16)  # silu(h1) packed 4ch/grp

            gsums = sm.tile([C, NGRP], f32)   # per-window sum of x
            ysum2 = sm.tile([C, 2 * NGRP], f32)  # per-half-group sum of y
            ysq = sm.tile([C, 2 * NGRP], f32)    # per-half-group sum of y^2

            sqdump = sqp.tile([C, NW], bf16, tag="sqd")
            dvsq = sqp.tile([C, 2 * NCH], bf16, tag="dvsq")

            # ---- ACT table prefetch (all sets used mid-run) ----
            scr1 = sm.tile([C, 1], f32)
            for fn in (Act.Copy, Act.Identity, Act.Square, Act.Silu):
                nc.scalar.activation(scr1, b1t, fn)

            # ---- x loads (sync ring) + gsum accum split ACT/DVE ----
            for g in range(NGRP):
                lo, hi = g * NW, (g + 1) * NW
                nc.sync.dma_start(out=xbf[:, lo:hi], in_=x_ext[:, lo:hi])
            for t, e in [(wcho, wcho_ext), (wchi, wchi_ext),
                         (wm1t, wm1t_ext), (taps, taps_ext),
                         (wa2t, wa2t_ext), (wdt, wdt_ext), (c2t4, c2t4_ext),
                         (bout, bout_ext), (gb, gb_ext)]:
                nc.sync.dma_start(out=t, in_=e[:])
            # HAM warmup: a dense ~3.4us burst right after x0 flips the PE
            # clock gate to 2.4GHz (scattered tiny matmuls never sustain a
            # full SHORT window), then keep-alives paced by the x arrivals
            # stop the MID window from re-throttling before the ramp.
            ham_ps = sps.tile([C, NCH], f32, tag="sp")
            for i in range(8):
                nc.tensor.matmul(ham_ps, wb3t, xbf[:, 0:NCH],
                                 start=True, stop=True)
            for g in range(1, 7):
                nc.tensor.matmul(ham_ps[:, 0:128], wb3t,
                                 xbf[:, g * NW:g * NW + 128],
                                 start=True, stop=True)
            for g in range(NGRP - 2):
                lo, hi = g * NW, (g + 1) * NW
                nc.scalar.activation(sqdump, xbf[:, lo:hi], Act.Copy,
                                     accum_out=gsums[:, g:g + 1])

            def gsum_chain():
                """gsum finalize (2 DVE window reduces right after windows0)
                and the channel-branch small chain. All small PSUM
                evacuations run on ACT so the DVE FIFO stays free for the
                windows pipeline; only the reciprocal stays on DVE and is
                issued before windows1 (it costs <1us of FIFO block)."""
                gsum = sm.tile([C, 1], f32)
                nc.vector.tensor_reduce(gsums[:, NGRP - 2:NGRP - 1],
                                        xbf[:, (NGRP - 2) * NW:
                                            (NGRP - 1) * NW],
                                        mybir.AxisListType.X, Alu.add)
                nc.vector.tensor_reduce(gsums[:, NGRP - 1:NGRP],
                                        xbf[:, (NGRP - 1) * NW:L],
                                        mybir.AxisListType.X, Alu.add)
                nc.vector.tensor_reduce(gsum, gsums, mybir.AxisListType.X,
                                        Alu.add)

                ss_ps = sps.tile([1, 1], f32, tag="sp")
                nc.tensor.matmul(ss_ps, gsum, gsum, start=True, stop=True)
                ss = sm.tile([1, 1], f32)
                nc.scalar.activation(ss, ss_ps, Act.Copy)
                rn2 = sm.tile([1, 1], f32)
                nc.vector.reciprocal(rn2, ss)          # 1 / ||gsum||^2

                v_ps = sps.tile([C, 1], f32, tag="sp")
                nc.tensor.matmul(v_ps, wcho, gsum, start=True, stop=True)
                v_sb = sm.tile([C, 1], f32)
                nc.scalar.activation(v_sb, v_ps, Act.Copy)
                pqr = sm.tile([C, 3], f32)
                nc.scalar.activation(pqr, taps, Act.Copy,
                                     scale=v_sb[:, 0:1])
                pqr2_ps = sps.tile([C, 3], f32, tag="sp")
                nc.tensor.matmul(pqr2_ps, wchi, pqr, start=True, stop=True)
                pqr2 = sm.tile([C, 3], f32)
                nc.scalar.activation(pqr2, pqr2_ps, Act.Copy)

                u_ps = sps.tile([1, 32], f32, tag="sp")
                nc.tensor.matmul(u_ps, gsum, wm1t, start=True, stop=True)
                u_sb = sm.tile([1, 32], f32)
                nc.scalar.activation(u_sb, u_ps, Act.Copy)
                u_sc = sm.tile([1, 32], f32)
                nc.scalar.activation(u_sc, u_sb, Act.Copy,
                                     scale=rn2[0:1, 0:1])
                # broadcast u across partitions on PE (ones outer product)
                ubc_ps = sps.tile([C, 32], f32, tag="sp")
                nc.tensor.matmul(ubc_ps, ones_row, u_sc, start=True,
                                 stop=True)
                u_bc = sm.tile([C, 32], f32)
                nc.scalar.activation(u_bc, ubc_ps, Act.Copy)

                mqt = sm.tile([C, 32], bf16)
                mpt = sm.tile([C, 32], bf16)
                mrt = sm.tile([C, 32], bf16)
                for t, j in [(mpt, 0), (mqt, 1), (mrt, 2)]:
                    nc.scalar.activation(t, u_bc, Act.Copy,
                                         scale=pqr2[:, j:j + 1])
                return mqt, mpt, mrt

            # ---- main pipeline over groups ----
            def windows_rest(g):
                """sh, sv, dh/|dh|/H, dv/|dv|/V for group g (DVE)."""
                G0 = g * NW
                sh = win.tile([C, NW], bf16, tag="sh")
                # s_h[t] = x[l-1] + x[l+1]
                ha = 1 if g == 0 else 0
                hb = NW - 1 if g == NGRP - 1 else NW
                nc.vector.tensor_tensor(sh[:, ha:hb],
                                        xbf[:, G0 + ha - 1:G0 + hb - 1],
                                        xbf[:, G0 + ha + 1:G0 + hb + 1],
                                        Alu.add)
                if g == 0:
                    nc.vector.tensor_copy(sh[:, 0:1], xbf[:, 1:2])
                if g == NGRP - 1:
                    nc.vector.tensor_copy(sh[:, NW - 1:NW],
                                          xbf[:, L - 2:L - 1])
                sv = win.tile([C, NW], bf16, tag="sv")
                # s_v[t] = x[l-128] + x[l+128]
                va = 128 if g == 0 else 0
                vb = NW - 128 if g == NGRP - 1 else NW
                nc.vector.tensor_tensor(sv[:, va:vb],
                                        xbf[:, G0 + va - 128:G0 + vb - 128],
                                        xbf[:, G0 + va + 128:G0 + vb + 128],
                                        Alu.add)
                if g == 0:
                    nc.vector.tensor_copy(sv[:, 0:128], xbf[:, 128:256])
                if g == NGRP - 1:
                    nc.vector.tensor_copy(sv[:, NW - 128:NW],
                                          xbf[:, L - 256:L - 128])

                dh = dwin.tile([C, NW + 4], bf16, tag="dh")
                dv = dwin.tile([C, NW + 128], bf16, tag="dv")
                Hw = win.tile([C, NW], bf16, tag="Hw")

                # d_h[t] = |x[G0+t] - x[G0+t-1]|, t in [a, e)
                a = 1 if g == 0 else 0
                e = NW if g == NGRP - 1 else NW + 1
                nc.vector.tensor_tensor(dh[:, a:e], xbf[:, G0 + a:G0 + e],
                                        xbf[:, G0 + a - 1:G0 + e - 1],
                                        Alu.subtract)
                if g == 0:
                    nc.vector.memset(dh[:, 0:1], 0.0)
                dhu = dh.bitcast(mybir.dt.uint16)
                nc.vector.tensor_scalar(dhu[:, 0:e], dhu[:, 0:e], 0x7FFF,
                                        None, Alu.bitwise_and)
                # H[t] = d_h[t] + d_h[t+1], edges fixed per image row
                he = NW if g < NGRP - 1 else NW - 1
                nc.vector.tensor_tensor(Hw[:, 0:he], dh[:, 0:he],
                                        dh[:, 1:he + 1], Alu.add)
                h3 = Hw.rearrange("p (r c) -> p r c", c=Wimg)
                d3 = dh[:, 0:NW].rearrange("p (r c) -> p r c", c=Wimg)
                nc.vector.tensor_scalar(h3[:, :, 0:1], d3[:, :, 1:2], 2.0,
                                        None, Alu.mult)
                nc.vector.tensor_scalar(h3[:, :, Wimg - 1:Wimg],
                                        d3[:, :, Wimg - 1:Wimg], 2.0, None,
                                        Alu.mult)

                # d_v[t] = |x[G0+t] - x[G0+t-128]|, t in [av, ev)
                av = 128 if g == 0 else 0
                ev = NW if g == NGRP - 1 else NW + 128
                nc.vector.tensor_tensor(dv[:, av:ev], xbf[:, G0 + av:G0 + ev],
                                        xbf[:, G0 + av - 128:G0 + ev - 128],
                                        Alu.subtract)
                dvu = dv.bitcast(mybir.dt.uint16)
                nc.vector.tensor_scalar(dvu[:, av:ev], dvu[:, av:ev], 0x7FFF,
                                        None, Alu.bitwise_and)
                # V[t] = d_v[t] + d_v[t+128], first/last image row fixed
                Vw = win.tile([C, NW], bf16, tag="Vw")
                vlo = 128 if g == 0 else 0
                vhi = NW - 128 if g == NGRP - 1 else NW
                nc.vector.tensor_tensor(Vw[:, vlo:vhi], dv[:, vlo:vhi],
                                        dv[:, vlo + 128:vhi + 128], Alu.add)
                if g == 0:
                    nc.vector.tensor_scalar(Vw[:, 0:128], dv[:, 128:256], 2.0,
                                            None, Alu.mult)
                if g == NGRP - 1:
                    nc.vector.tensor_scalar(Vw[:, NW - 128:NW],
                                            dv[:, NW - 128:NW], 2.0, None,
                                            Alu.mult)
                return sh, sv, Hw, Vw

            def channel_mms(k):
                """h1 psum for group k: 3 shifted matmuls x 4 col-bands."""
                h1ps = hps.tile([C, NCH], f32)
                for wgt, shift in [(mqt, 0), (mpt, -1), (mrt, +1)]:
                    for j in range(4):
                        n = 4 * k + j
                        n0 = n * NCH
                        lo = n0 + shift
                        hi = n0 + NCH + shift
                        plo, phi = 0, NCH
                        if lo < 0:
                            plo, lo = 1, 0
                        if hi > L:
                            phi, hi = NCH - 1, L
                        nc.tensor.matmul(
                            h1ps[32 * j:32 * j + 32, plo:phi],
                            wgt[:, 0:32], xbf[:, lo:hi],
                            start=(shift == 0), stop=(shift == 1),
                            tile_position=(0, 32 * j))
                return h1ps

            def main_half(k, h, sh, sv, Hw, Vw):
                """Half-group h (2 chunks) of group k: weight-outer matmuls
                into one [C, 1024] psum tile, then one ACT evacuation.

                sh (x[l-1]+x[l+1]) is realized as two shifted matmuls on x;
                the boundary clip reproduces the one-sided edges exactly."""
                ns = [4 * k + 2 * h, 4 * k + 2 * h + 1]
                ps = yps.tile([C, 2 * NCH], f32)
                off = [0, NCH]              # chunk base col inside ps
                woff = 2 * h * NCH          # chunk base col inside window

                for i, n in enumerate(ns):
                    nc.tensor.matmul(ps[:, off[i]:off[i] + NCH], wb3t,
                                     xbf[:, n * NCH:(n + 1) * NCH],
                                     start=True, stop=False)
                for i, n in enumerate(ns):
                    o = woff + i * NCH
                    nc.tensor.matmul(ps[:, off[i]:off[i] + NCH], wa2t,
                                     sh[:, o:o + NCH],
                                     start=False, stop=False)
                for i, n in enumerate(ns):
                    o = woff + i * NCH
                    nc.tensor.matmul(ps[:, off[i]:off[i] + NCH], wa2t,
                                     sv[:, o:o + NCH],
                                     start=False, stop=False)
                    if n == 0:
                        # col-scan wrap: l=j gets x[(h-1)w + j - 1]
                        nc.tensor.matmul(ps[:, off[i] + 1:off[i] + 128], wa2t,
                                         xbf[:, L - Wimg:L - 1],
                                         start=False, stop=False)
                    if n == NCHUNK - 1:
                        # col-scan wrap: l=(h-1)w+j gets x[j+1]
                        nc.tensor.matmul(
                            ps[:, off[i] + NCH - 128:off[i] + NCH - 1], wa2t,
                            xbf[:, 1:128], start=False, stop=False)
                for i, n in enumerate(ns):
                    o = woff + i * NCH
                    nc.tensor.matmul(ps[:, off[i]:off[i] + NCH], wdt,
                                     Hw[:, o:o + NCH], start=False, stop=False)
                for i, n in enumerate(ns):
                    o = woff + i * NCH
                    nc.tensor.matmul(ps[:, off[i]:off[i] + NCH], wdt,
                                     Vw[:, o:o + NCH], start=False, stop=False)
                for i, n in enumerate(ns):
                    j = n % 4
                    nc.tensor.matmul(
                        ps[:, off[i]:off[i] + NCH],
                        c2t4[32 * j:32 * j + 32, :],
                        h1sb[32 * j:32 * j + 32, k * NCH:(k + 1) * NCH],
                        start=False, stop=True, tile_position=(32 * j, 0))

                hidx = 2 * k + h
                n0 = ns[0] * NCH
                nc.scalar.activation(ypre[:, n0:n0 + 2 * NCH], ps,
                                     Act.Identity, bias=bout[:, 0:1],
                                     accum_out=ysum2[:, hidx:hidx + 1])
                if hidx >= 12:
                    # DVE is idle once its windows are done; give it the
                    # last groups' sum-of-squares to unclog the ACT tail
                    nc.vector.scalar_tensor_tensor(
                        dvsq, ypre[:, n0:n0 + 2 * NCH], 1.0,
                        ypre[:, n0:n0 + 2 * NCH], Alu.bypass, Alu.mult,
                        accum_out=ysq[:, hidx:hidx + 1])
                else:
                    nc.scalar.activation(sqdump[:, 0:2 * NCH],
                                         ypre[:, n0:n0 + 2 * NCH], Act.Square,
                                         accum_out=ysq[:, hidx:hidx + 1])

            def do_main(kk):
                sh, sv, Hw, Vw = wins.pop(kk)
                main_half(kk, 0, sh, sv, Hw, Vw)
                main_half(kk, 1, sh, sv, Hw, Vw)

            wins = {}
            h1s = {}
            wins[0] = windows_rest(0)
            mqt, mpt, mrt = gsum_chain()
            for k in range(NGRP):
                if k >= 1:
                    wins[k] = windows_rest(k)
                h1s[k] = channel_mms(k)
                # evacs of group k-2 are issued BEFORE silu_k: the strict
                # ACT FIFO must not queue them behind a silu that itself
                # waits on PE, or the next group's PSUM reuse stalls PE
                if k >= 2:
                    do_main(k - 2)
                nc.scalar.activation(h1sb[:, k * NCH:(k + 1) * NCH], h1s[k],
                                     Act.Silu, bias=b1t[:, 0:1])
            do_main(NGRP - 2)
            do_main(NGRP - 1)

            # Sqrt table prefetch: loads while the collective runs
            nc.scalar.activation(scr1, b1t, Act.Sqrt)

            # ---- global BN stats via AllReduce ----
            stats = sm.tile([C, 2], f32)
            nc.vector.tensor_reduce(stats[:, 0:1], ysum2, mybir.AxisListType.X,
                                    Alu.add)
            nc.vector.tensor_reduce(stats[:, 1:2], ysq, mybir.AxisListType.X,
                                    Alu.add)
            cc_in = dram.tile([C, 2], f32)
            cc_out = dram.tile([C, 2], f32)
            nc.sync.dma_start(out=cc_in[:], in_=stats)
            nc.gpsimd.collective_compute(
                "AllReduce", Alu.add,
                replica_groups=[list(range(NCORES))],
                ins=[cc_in.opt()], outs=[cc_out.opt()])
            statsr = sm.tile([C, 2], f32)
            nc.sync.dma_start(out=statsr, in_=cc_out[:])

            moments = sm.tile([C, 2], f32)   # [mean, E(y^2)]
            nc.vector.tensor_scalar(moments, statsr, 1.0 / NTOT, None,
                                    Alu.mult)
            m2 = sm.tile([C, 1], f32)
            nc.vector.tensor_tensor(m2, moments[:, 0:1], moments[:, 0:1],
                                    Alu.mult)
            varep = sm.tile([C, 1], f32)
            nc.vector.tensor_tensor(varep, moments[:, 1:2], m2, Alu.subtract)
            nc.vector.tensor_scalar(varep, varep, EPS_BN, None, Alu.add)
            inv = sm.tile([C, 1], f32)
            nc.vector.reciprocal(inv, varep)
            rstd = sm.tile([C, 1], f32)
            nc.scalar.activation(rstd, inv, Act.Sqrt)
            s_sc = sm.tile([C, 1], f32)
            nc.vector.tensor_tensor(s_sc, rstd, gb[:, 0:1], Alu.mult)
            ms = sm.tile([C, 1], f32)
            nc.vector.tensor_tensor(ms, moments[:, 0:1], s_sc, Alu.mult)
            t_sc = sm.tile([C, 1], f32)
            nc.vector.tensor_tensor(t_sc, gb[:, 1:2], ms, Alu.subtract)

            # ---- apply BN (4 double-width 4x-mode passes), write out
            # bf16 on two HWDGE rings ----
            for g in range(NGRP // 2):
                lo, hi = g * 2 * NW, (g + 1) * 2 * NW
                ow = owp.tile([C, 2 * NW], bf16, tag="ow")
                nc.vector.tensor_scalar(ow, ypre[:, lo:hi],
                                        s_sc[:, 0:1], t_sc[:, 0:1],
                                        Alu.mult, Alu.add)
                eng = nc.sync if g % 2 == 0 else nc.scalar
                eng.dma_start(out=y_ext[:, lo:hi], in_=ow)

    _split_excess_waits(nc)
    return nc


def _fold_weights(inputs):
    f = np.float32
    W_in = inputs["w_spatial_in"].astype(np.float64)
    W_out = inputs["w_spatial_out"].astype(np.float64)
    dw_sp = inputs["w_dw_spatial"][:, 0, :].astype(np.float64)
    W_proj = inputs["w_out_proj"].astype(np.float64)
    W_mlp2 = inputs["w_mlp2"].astype(np.float64)
    dwt = float(inputs["diff_weight"])

    a_sym = dw_sp[:, 0] + dw_sp[:, 2]
    w1 = dw_sp[:, 1]
    A2 = 0.25 * W_proj @ (W_out * a_sym[None, :]) @ W_in
    B3 = W_proj @ (W_out * w1[None, :]) @ W_in + W_proj
    W_d = 0.25 * dwt * W_proj
    C2 = W_proj @ W_mlp2                     # [c, 32]
    bias_out = W_proj @ inputs["b_mlp2"].astype(np.float64)

    bf = ml_dtypes.bfloat16
    return {
        "wb3t": np.ascontiguousarray(B3.T.astype(bf)),
        "wa2t": np.ascontiguousarray(A2.T.astype(bf)),
        "wdt": np.ascontiguousarray(W_d.T.astype(bf)),
        "c2t4": np.ascontiguousarray(np.tile(C2.T.astype(bf), (4, 1))),
        "wcho": np.ascontiguousarray(inputs["w_ch_out"].astype(f)),
        "wchi": np.ascontiguousarray(inputs["w_ch_in"].astype(f)),
        "wm1t": np.ascontiguousarray(inputs["w_mlp1"].T.astype(f)),
        "taps": np.ascontiguousarray(inputs["w_ch_dw"][:, 0, :].astype(f)),
        "b1t": np.ascontiguousarray(
            np.tile(inputs["b_mlp1"].astype(f), 4)[:, None]),
        "bout": np.ascontiguousarray(bias_out.astype(f)[:, None]),
        "gb": np.ascontiguousarray(
            np.stack([inputs["bn_gamma"], inputs["bn_beta"]], 1).astype(f)),
    }


def _make_in_maps(inputs):
    wmap = _fold_weights(inputs)
    x = inputs["x"].astype(np.float32)  # [B, C, H, W]
    in_maps = []
    for b in range(NCORES):
        m = dict(wmap)
        m["x"] = np.ascontiguousarray(
            x[b].reshape(C, L).astype(ml_dtypes.bfloat16))
        in_maps.append(m)
    return in_maps


def kernel(**inputs):
    from concourse.bass_utils import run_bass_kernel_spmd

    inputs = {k: np.asarray(v) for k, v in inputs.items()}
    if "nc" not in _CACHE:
        _CACHE["nc"] = _build_program()
    nc = _CACHE["nc"]

    in_maps = _make_in_maps(inputs)
    res = run_bass_kernel_spmd(nc, in_maps, list(range(NCORES)))
    out = np.stack([np.asarray(res.results[b]["y"])
                    .astype(np.float32).reshape(C, Himg, Wimg)
                    for b in range(NCORES)])
    return out
